# revision 6
# baseline (speedup 1.0000x reference)
"""Trainium2 Bass kernel for nn_MixtureOfAdapter (moe_routing).

Math (per token, H=1024, F=256, D=3 domains):
    mu, sd (ddof=1) over H;  s = sd + eps;  xn = (x - mu)/s
    h_d   = xn*g_d + b_d
    mid_d = relu(W1_d h_d + b1_d);  a_d = W2_d mid_d + b2_d
    gate_d = sigmoid(gu_d.x + gv_d.a_d + gb_d)
    out = 2x + sum_d gate_d * a_d

Kernel strategy (8 cores, data-parallel over batch B=8):
  - All matmul-land tensors are bf16: transposes and matmuls run at
    1 cyc/row on the PE, and weights/activations halve SBUF + DMA.
  - Work in normalized-transposed land: per 512-token macro-tile the
    centered/normalized xn = (x-mu)/s (computed by one Activation
    Identity op with per-partition scale=1/s, bias=-mu/s) is moved to
    [h, t] layout by the DMA XBAR transpose (dma_start_transpose), not
    the PE.  Two extra bf16 columns (mu, s) ride along in the same
    transpose and come out as [1, t] rows for rank-1 corrections.
  - M1: mid = relu(W1g @ xn^T (+ b1e per-partition bias)) with
    W1g = W1 * ln_g folded host-side.  True mid (no s scaling).
  - Gates: pgv[d,t] = w2gv_d . mid_d (+ mu-row rank-1 for gu.x's mean
    term); pgux[d,t] = gu_d . xn^T; z = pgux*s + pgv;
    gate = sigmoid(z + (gb_d + gv_d.b2_d)).  s broadcast to 3
    partitions via a ones3 rank-1 matmul.
  - gate rows broadcast to 128 partitions via one-hot matmuls; gmid =
    mid * gate (bf16, 2x DVE); M2 accumulates all domains into one
    PSUM in natural [t, h] layout (+ gate-row rank-1 if b2 nonzero);
    out = 2x + pout via one DVE scalar_tensor_tensor per 512-chunk.
  - Software-pipelined emission keeps each macro-tile's gate chain
    (DVE/Act latency) hidden behind the next tile's M1 in the PE FIFO;
    macro-tile 0's M1 is emitted in 128-token slices so the PE starts
    as soon as the first sub-tile's transpose lands.
"""

import numpy as np

import concourse.bass as bass
import concourse.mybir as mybir
import concourse.tile as tile
from concourse.bass_utils import run_bass_kernel_spmd

B, L, H, F, D = 8, 2048, 1024, 256, 3
EPS = 1e-6
T = 512                 # tokens per macro-tile
NSUB = T // 128         # 4 sub-tiles of 128 tokens
NMT = L // T            # 4 macro-tiles per core
KCH = H // 128          # 8 k-chunks over H
FCH = (D * F) // 128    # 6 chunks over stacked (domain, F)
NCH = H // 512          # 2 output column chunks
DF = D * F
XW = H + 128            # transpose width: H cols + (mu, s, pad) block

f32 = mybir.dt.float32
bf16 = mybir.dt.bfloat16
AF = mybir.ActivationFunctionType
ALU = mybir.AluOpType


def _split_multiwaits(nc):
    """This walrus build allows 1 sync-wait per instruction (2 for
    EventSemaphore); Tile can attach more.  Move extras onto preceding
    same-engine NoOps (engine queues are FIFO, so semantics identical)."""
    for f in nc.m.functions:
        for bb in f.blocks:
            new = []
            changed = False
            for inst in bb.instructions:
                si = inst.sync_info
                cap = 2 if isinstance(inst, mybir.InstEventSemaphore) else 1
                if si is not None and len(si.on_wait) > cap:
                    waits = list(si.on_wait)
                    extra, kept = waits[:-cap], waits[-cap:]
                    for j, w in enumerate(extra):
                        new.append(mybir.InstNoOp(
                            name=f"{inst.name}-wsplit{j}",
                            engine=inst.engine,
                            sync_info=mybir.SyncInfo(on_wait=[w], on_update=[]),
                            ins=[], outs=[],
                        ))
                    inst.sync_info = mybir.SyncInfo(
                        on_wait=kept, on_update=list(si.on_update))
                    changed = True
                new.append(inst)
            if changed:
                bb.instructions = new


def _build(has_b1e: bool, has_b2: bool):
    nc = bass.Bass(target_bir_lowering=False)

    xin = nc.dram_tensor("xin", [L, H], f32, kind="ExternalInput")
    w1g = nc.dram_tensor("w1g", [128, KCH, DF], bf16, kind="ExternalInput")
    w2t = nc.dram_tensor("w2t", [128, FCH, H], bf16, kind="ExternalInput")
    cpb = nc.dram_tensor("cpb", [128, 432], bf16, kind="ExternalInput")
    cpf = nc.dram_tensor("cpf", [128, 8], f32, kind="ExternalInput")
    if has_b2:
        b2r = nc.dram_tensor("b2r", [D, H], bf16, kind="ExternalInput")
    out = nc.dram_tensor("out", [L, H], f32, kind="ExternalOutput")

    # [L, H] viewed as [128p, sub, H] per macro-tile
    x_mt = xin.ap().rearrange("(m s p) h -> m p s h", p=128, s=NSUB)
    out_mt = out.ap().rearrange("(m s p) h -> m p s h", p=128, s=NSUB)

    with tile.TileContext(nc) as tc:
        with (
            tc.tile_pool(name="const", bufs=1) as const,
            tc.tile_pool(name="xp", bufs=2) as xp,
            tc.tile_pool(name="xnp", bufs=2) as xnp,
            tc.tile_pool(name="xtp", bufs=2) as xtp,
            tc.tile_pool(name="midp", bufs=2) as midp,
            tc.tile_pool(name="gmp", bufs=2) as gmp,
            tc.tile_pool(name="gbp", bufs=2) as gbp,
            tc.tile_pool(name="outp", bufs=2) as outp,
            tc.tile_pool(name="smalls", bufs=3) as smalls,
            tc.tile_pool(name="gsm", bufs=2) as gsm,
            tc.tile_pool(name="ps_m1", bufs=2, space="PSUM") as ps_m1,
            tc.tile_pool(name="ps_m2", bufs=2, space="PSUM") as ps_m2,
            tc.tile_pool(name="ps_gux", bufs=1, space="PSUM") as ps_gux,
            tc.tile_pool(name="ps_gv", bufs=1, space="PSUM") as ps_gv,
            tc.tile_pool(name="ps_bc", bufs=2, space="PSUM") as ps_bc,
        ):
            # constants + first x macro-tile; x on sync queue, weights on
            # scalar queue so both HWDGE streams fill in parallel
            cpb_sb = const.tile([128, 432], bf16)
            cpf_sb = const.tile([128, 8], f32)
            nc.scalar.dma_start(out=cpb_sb, in_=cpb.ap())
            nc.scalar.dma_start(out=cpf_sb, in_=cpf.ap())
            oh_sb = cpb_sb[0:D, 0:384]
            gus_sb = cpb_sb[:, 384:408].rearrange("p (k d) -> p k d", d=D)
            w2gv_sb = cpb_sb[:, 408:426].rearrange("p (c d) -> p c d", d=D)
            gusum_sb = cpb_sb[0:1, 426:429]
            ones3_sb = cpb_sb[32:33, 429:432]
            gb3_sb = cpf_sb[0:D, 6:7]

            x_first = xp.tile([128, NSUB, H], f32, tag="x")
            w1g_sb = const.tile([128, KCH, DF], bf16)
            for ss in range(NSUB):
                nc.sync.dma_start(out=x_first[:, ss, :], in_=x_mt[0][:, ss, :])
                nc.scalar.dma_start(out=w1g_sb[:, ss * 2, :],
                                    in_=w1g.ap()[:, ss * 2, :])
                nc.scalar.dma_start(out=w1g_sb[:, ss * 2 + 1, :],
                                    in_=w1g.ap()[:, ss * 2 + 1, :])
            w2t_sb = const.tile([128, FCH, H], bf16)
            for c in range(2):
                nc.scalar.dma_start(out=w2t_sb[:, c * 3:(c + 1) * 3, :],
                                    in_=w2t.ap()[:, c * 3:(c + 1) * 3, :])
            if has_b2:
                b2r_sb = const.tile([D, H], bf16)
                nc.scalar.dma_start(out=b2r_sb, in_=b2r.ap())

            def stage_a(mt, x_pre=None):
                """x load, stats, normalize (bf16), DMA-transpose."""
                if x_pre is not None:
                    x_t = x_pre
                else:
                    x_t = xp.tile([128, NSUB, H], f32, tag="x")
                    for ss in range(NSUB):
                        nc.sync.dma_start(out=x_t[:, ss, :],
                                          in_=x_mt[mt][:, ss, :])
                xn_b = xnp.tile([128, NSUB, XW], bf16, tag="xn")
                xnT = xtp.tile([128, KCH + 1, T], bf16, tag="xnT")
                for ss in range(NSUB):
                    xs = x_t[:, ss, :]
                    st = smalls.tile([128, 2, 6], f32, tag="bnst")
                    nc.vector.bn_stats(out=st[:, 0, :], in_=xs[:, 0:512])
                    nc.vector.bn_stats(out=st[:, 1, :], in_=xs[:, 512:1024])
                    mv = smalls.tile([128, 2], f32, tag="mv")
                    nc.vector.bn_aggr(out=mv, in_=st)
                    # sc: 0=r=1/s, 1=-mu*r, 2=s=sd+eps
                    sc = smalls.tile([128, 4], f32, tag="sc")
                    nc.scalar.activation(out=sc[:, 2:3], in_=mv[:, 1:2],
                                         func=AF.Sqrt,
                                         scale=float(H) / (H - 1))
                    nc.vector.tensor_scalar_add(sc[:, 2:3], sc[:, 2:3], EPS)
                    nc.vector.reciprocal(sc[:, 0:1], sc[:, 2:3])
                    nc.vector.tensor_scalar(out=sc[:, 1:2], in0=mv[:, 0:1],
                                            scalar1=sc[:, 0:1], scalar2=-1.0,
                                            op0=ALU.mult, op1=ALU.mult)
                    # mu, s ride along in the transpose as bf16 columns;
                    # col H -> row partition 0, col H+32 -> partition 32
                    # (matmul base partitions must be 0/32/64)
                    nc.vector.tensor_copy(xn_b[:, ss, H:H + 1], mv[:, 0:1])
                    nc.vector.tensor_copy(xn_b[:, ss, H + 32:H + 33],
                                          sc[:, 2:3])
                    # xn = x*(1/s) + (-mu/s), one Activation op, bf16 out
                    nc.scalar.activation(out=xn_b[:, ss, 0:H], in_=xs,
                                         func=AF.Identity,
                                         scale=sc[:, 0:1], bias=sc[:, 1:2])
                    nc.scalar.dma_start_transpose(
                        xnT[:, :, ss * 128:(ss + 1) * 128], xn_b[:, ss, :])
                return dict(x_t=x_t, xnT=xnT)

            def stage_b(mt, st_, sliced=False):
                """M1: mid = relu(W1g @ xn^T (+ b1e))."""
                xnT = st_["xnT"]
                mid = midp.tile([128, FCH, T], bf16, tag="mid")
                for c in range(FCH):
                    p1 = ps_m1.tile([128, T], f32, tag="m1")
                    w1c = w1g_sb[:, :, c * 128:(c + 1) * 128]
                    if sliced:
                        for ss in range(NSUB):
                            tsl = slice(ss * 128, (ss + 1) * 128)
                            for k in range(KCH):
                                nc.tensor.matmul(
                                    p1[:, tsl], w1c[:, k, :], xnT[:, k, tsl],
                                    start=(k == 0), stop=(k == KCH - 1))
                    else:
                        for k in range(KCH):
                            nc.tensor.matmul(p1, w1c[:, k, :], xnT[:, k, :],
                                             start=(k == 0),
                                             stop=(k == KCH - 1))
                    bias = cpf_sb[:, c:c + 1] if has_b1e else 0.0
                    nc.scalar.activation(out=mid[:, c, :], in_=p1,
                                         func=AF.Relu, bias=bias)
                st_["mid"] = mid

            def stage_c(mt, st_):
                """Gates + gmid = gate * mid."""
                xnT, mid = st_["xnT"], st_["mid"]
                murow = xnT[0:1, KCH, :]
                srow = xnT[32:33, KCH, :]
                pgux = ps_gux.tile([D, T], f32, tag="gux")
                for k in range(KCH):
                    nc.tensor.matmul(pgux, gus_sb[:, k, :], xnT[:, k, :],
                                     start=(k == 0), stop=(k == KCH - 1))
                pgv = ps_gv.tile([D, T], f32, tag="gv")
                for c in range(FCH):
                    nc.tensor.matmul(pgv, w2gv_sb[:, c, :], mid[:, c, :],
                                     start=(c == 0), stop=False)
                # gu.x = s*(gu.xn) + mu*sum(gu): mu rank-1 joins pgv's psum
                nc.tensor.matmul(pgv, gusum_sb, murow, start=False, stop=True)
                # s broadcast to 3 partitions
                s3_ps = ps_bc.tile([D, T], f32, tag="bc")
                nc.tensor.matmul(s3_ps, ones3_sb, srow, start=True, stop=True)
                s3b = gsm.tile([D, T], bf16, tag="s3")
                nc.scalar.activation(out=s3b, in_=s3_ps, func=AF.Copy)
                z_sb = gsm.tile([D, T], f32, tag="z")
                nc.vector.tensor_tensor(out=z_sb, in0=pgux, in1=s3b,
                                        op=ALU.mult)
                nc.vector.tensor_add(z_sb, z_sb, pgv)
                g_t = gsm.tile([D, T], bf16, tag="g")
                nc.scalar.activation(out=g_t, in_=z_sb, func=AF.Sigmoid,
                                     bias=gb3_sb)
                # broadcast gate row d across partitions via one-hot matmul
                gb128 = gbp.tile([128, D, T], bf16, tag="gb")
                for d in range(D):
                    p_b = ps_bc.tile([128, T], f32, tag="bc")
                    nc.tensor.matmul(p_b, oh_sb[:, d * 128:(d + 1) * 128],
                                     g_t, start=True, stop=True)
                    nc.scalar.activation(out=gb128[:, d, :], in_=p_b,
                                         func=AF.Copy)
                gmid = gmp.tile([128, FCH, T], bf16, tag="gmid")
                for c in range(FCH):
                    nc.vector.tensor_mul(gmid[:, c, :], mid[:, c, :],
                                         gb128[:, c // 2, :])
                st_["gmid"] = gmid
                st_["g_t"] = g_t

            def stage_d(mt, st_):
                """M2 accumulates all domains (+gate*b2) + final out."""
                gmid, x_t = st_["gmid"], st_["x_t"]
                for ss in range(NSUB):
                    tsl = slice(ss * 128, (ss + 1) * 128)
                    out_sb = outp.tile([128, H], f32, tag="osb")
                    for nch in range(NCH):
                        hsl = slice(nch * 512, (nch + 1) * 512)
                        po = ps_m2.tile([128, 512], f32, tag="m2")
                        for c in range(FCH):
                            nc.tensor.matmul(po, gmid[:, c, tsl],
                                             w2t_sb[:, c, hsl],
                                             start=(c == 0),
                                             stop=(c == FCH - 1 and not has_b2))
                        if has_b2:
                            nc.tensor.matmul(po, st_["g_t"][:, tsl],
                                             b2r_sb[:, hsl],
                                             start=False, stop=True)
                        # out = 2*x + pout
                        nc.vector.scalar_tensor_tensor(
                            out=out_sb[:, hsl], in0=x_t[:, ss, hsl],
                            scalar=2.0, in1=po, op0=ALU.mult, op1=ALU.add)
                    nc.sync.dma_start(out=out_mt[mt][:, ss, :], in_=out_sb)

            # software-pipelined emission: each macro-tile's gate chain
            # overlaps the next tile's M1 in the PE FIFO; mt0's M1 is
            # sliced so the PE starts on the first transposed sub-tile
            S = [None] * NMT
            S[0] = stage_a(0, x_pre=x_first)
            stage_b(0, S[0], sliced=True)
            S[1] = stage_a(1)
            stage_c(0, S[0])
            stage_b(1, S[1])
            stage_d(0, S[0])
            S[2] = stage_a(2)
            stage_c(1, S[1])
            stage_b(2, S[2])
            stage_d(1, S[1])
            S[3] = stage_a(3)
            stage_c(2, S[2])
            stage_b(3, S[3])
            stage_d(2, S[2])
            stage_c(3, S[3])
            stage_d(3, S[3])

    _split_multiwaits(nc)
    return nc


last_results = None

_built = {}


def _get_nc(has_b1e, has_b2):
    key = (has_b1e, has_b2)
    if key not in _built:
        _built[key] = _build(*key)
    return _built[key]


def _to_bf16(a):
    from ml_dtypes import bfloat16
    return np.asarray(a, dtype=np.float32).astype(bfloat16)


def kernel(x, ln_g, ln_b, W1, b1, W2, b2, gu, gv, gb):
    x = np.asarray(x, dtype=np.float32)
    ln_g = np.asarray(ln_g, dtype=np.float32)
    ln_b = np.asarray(ln_b, dtype=np.float32)
    W1 = np.asarray(W1, dtype=np.float32)
    b1 = np.asarray(b1, dtype=np.float32)
    W2 = np.asarray(W2, dtype=np.float32)
    b2 = np.asarray(b2, dtype=np.float32)
    gu = np.asarray(gu, dtype=np.float32)
    gv = np.asarray(gv, dtype=np.float32)
    gb = np.asarray(gb, dtype=np.float32)

    # ---- host precompute (all small: ~D*F*H) ----
    W1G = W1 * ln_g[:, None, :]                                # [D, F, H]
    b1e = b1 + np.einsum('dfh,dh->df', W1, ln_b)               # [D, F]
    w2gv = np.einsum('dh,dhf->df', gv, W2)                     # [D, F]
    gusum = gu.sum(axis=1)                                     # [D]
    gb_eff = gb + np.einsum('dh,dh->d', gv, b2)                # [D]

    has_b1e = bool(np.any(b1e != 0.0))
    has_b2 = bool(np.any(b2 != 0.0))

    # lhsT for M1: [128, KCH, DF]; col c*128+j = W1G[d(c), fh(c)*128+j, h]
    w1g_in = np.zeros((128, KCH, DF), dtype=np.float32)
    for c in range(FCH):
        d, fh = c // 2, c % 2
        w1g_in[:, :, c * 128:(c + 1) * 128] = (
            W1G[d].T.reshape(KCH, 128, F)[:, :, fh * 128:(fh + 1) * 128]
            .transpose(1, 0, 2))
    # W2 rhs for M2: [128, FCH, H]; w2t[p, c, h] = W2[d, h, fh*128+p]
    w2t_in = np.zeros((128, FCH, H), dtype=np.float32)
    for c in range(FCH):
        d, fh = c // 2, c % 2
        w2t_in[:, c, :] = W2[d, :, fh * 128:(fh + 1) * 128].T

    cpb_in = np.zeros((128, 432), dtype=np.float32)
    for d in range(D):
        cpb_in[d, d * 128:(d + 1) * 128] = 1.0                 # one-hot bcast
    cpb_in[:, 384:408] = np.ascontiguousarray(
        gu.T.reshape(KCH, 128, D).transpose(1, 0, 2)).reshape(128, KCH * D)
    w2gv_in = np.zeros((128, FCH, D), dtype=np.float32)
    for c in range(FCH):
        d, fh = c // 2, c % 2
        w2gv_in[:, c, d] = w2gv[d, fh * 128:(fh + 1) * 128]
    cpb_in[:, 408:426] = w2gv_in.reshape(128, FCH * D)
    cpb_in[0, 426:429] = gusum
    cpb_in[32, 429:432] = 1.0

    cpf_in = np.zeros((128, 8), dtype=np.float32)
    if has_b1e:
        for c in range(FCH):
            d, fh = c // 2, c % 2
            cpf_in[:, c] = b1e[d, fh * 128:(fh + 1) * 128]
    cpf_in[0:D, 6] = gb_eff

    nc = _get_nc(has_b1e, has_b2)

    common = {
        "w1g": _to_bf16(w1g_in),
        "w2t": _to_bf16(w2t_in),
        "cpb": _to_bf16(cpb_in),
        "cpf": cpf_in,
    }
    if has_b2:
        common["b2r"] = _to_bf16(b2)
    in_maps = [dict(common, xin=np.ascontiguousarray(x[c]))
               for c in range(B)]
    res = run_bass_kernel_spmd(nc, in_maps, core_ids=list(range(B)))
    global last_results
    last_results = res
    return np.stack([res.results[c]["out"] for c in range(B)])


# revision 15
# speedup vs baseline: 1.0396x; 1.0396x over previous
"""Trainium2 Bass kernel for nn_MixtureOfAdapter (moe_routing).

Math (per token, H=1024, F=256, D=3 domains):
    mu, sd (ddof=1) over H;  s = sd + eps;  xn = (x - mu)/s
    h_d   = xn*g_d + b_d
    mid_d = relu(W1_d h_d + b1_d);  a_d = W2_d mid_d + b2_d
    gate_d = sigmoid(gu_d.x + gv_d.a_d + gb_d)
    out = 2x + sum_d gate_d * a_d

Kernel strategy (8 cores, data-parallel over batch B=8):
  - All matmul-land tensors are bf16: transposes and matmuls run at
    1 cyc/row on the PE, and weights/activations halve SBUF + DMA.
  - Work in normalized-transposed land: per 512-token macro-tile the
    centered/normalized xn = (x-mu)/s (computed by one Activation
    Identity op with per-partition scale=1/s, bias=-mu/s) is moved to
    [h, t] layout by the DMA XBAR transpose (dma_start_transpose), not
    the PE.  Two extra bf16 columns (mu, s) ride along in the same
    transpose and come out as [1, t] rows for rank-1 corrections.
  - M1: mid = relu(W1g @ xn^T (+ b1e per-partition bias)) with
    W1g = W1 * ln_g folded host-side.  True mid (no s scaling).
  - Gates: pgv[d,t] = w2gv_d . mid_d (+ mu-row rank-1 for gu.x's mean
    term); pgux[d,t] = gu_d . xn^T; z = pgux*s + pgv;
    gate = sigmoid(z + (gb_d + gv_d.b2_d)).  s broadcast to 3
    partitions via a ones3 rank-1 matmul.
  - gate rows broadcast to 128 partitions via one-hot matmuls; gmid =
    mid * gate (bf16, 2x DVE); M2 accumulates all domains into one
    PSUM in natural [t, h] layout (+ gate-row rank-1 if b2 nonzero);
    out = 2x + pout via one DVE scalar_tensor_tensor per 512-chunk.
  - Software-pipelined emission keeps each macro-tile's gate chain
    (DVE/Act latency) hidden behind the next tile's M1 in the PE FIFO;
    macro-tile 0's M1 is emitted in 128-token slices so the PE starts
    as soon as the first sub-tile's transpose lands.
"""

import numpy as np

import concourse.bass as bass
import concourse.mybir as mybir
import concourse.tile as tile
from concourse.bass_utils import run_bass_kernel_spmd

B, L, H, F, D = 8, 2048, 1024, 256, 3
EPS = 1e-6
T = 512                 # tokens per macro-tile
NSUB = T // 128         # 4 sub-tiles of 128 tokens
NMT = L // T            # 4 macro-tiles per core
KCH = H // 128          # 8 k-chunks over H
FCH = (D * F) // 128    # 6 chunks over stacked (domain, F)
NCH = H // 512          # 2 output column chunks
DF = D * F
XW = H + 128            # transpose width: H cols + (mu, s, pad) block

f32 = mybir.dt.float32
bf16 = mybir.dt.bfloat16
AF = mybir.ActivationFunctionType
ALU = mybir.AluOpType


def _split_multiwaits(nc):
    """This walrus build allows 1 sync-wait per instruction (2 for
    EventSemaphore); Tile can attach more.  Move extras onto preceding
    same-engine carrier instructions.  A bare NoOp holds the sequencer
    while it waits (stalling dispatch of everything behind it), so where
    possible the carrier is a 1-element Memset to a dead scratch column:
    a real engine instruction parks its wait in the engine wait queue
    and lets the sequencer keep dispatching."""
    import copy
    tmpl = {}
    for f in nc.m.functions:
        for bb in f.blocks:
            for inst in bb.instructions:
                if (isinstance(inst, mybir.InstMemset)
                        and inst.engine not in tmpl):
                    tmpl[inst.engine] = inst
                elif (isinstance(inst, mybir.InstActivation)
                        and inst.func == AF.Copy
                        and inst.engine not in tmpl):
                    tmpl[inst.engine] = inst

    def carrier(inst, w, j):
        t = tmpl.get(inst.engine)
        if t is not None:
            c = copy.deepcopy(t)
            c.name = f"{inst.name}-wsplit{j}"
            c.sync_info = mybir.SyncInfo(on_wait=[w], on_update=[])
            return c
        return mybir.InstNoOp(
            name=f"{inst.name}-wsplit{j}",
            engine=inst.engine,
            sync_info=mybir.SyncInfo(on_wait=[w], on_update=[]),
            ins=[], outs=[],
        )

    for f in nc.m.functions:
        for bb in f.blocks:
            new = []
            changed = False
            for inst in bb.instructions:
                si = inst.sync_info
                cap = 2 if isinstance(inst, mybir.InstEventSemaphore) else 1
                if si is not None and len(si.on_wait) > cap:
                    waits = list(si.on_wait)
                    extra, kept = waits[:-cap], waits[-cap:]
                    for j, w in enumerate(extra):
                        new.append(carrier(inst, w, j))
                    inst.sync_info = mybir.SyncInfo(
                        on_wait=kept, on_update=list(si.on_update))
                    changed = True
                new.append(inst)
            if changed:
                bb.instructions = new


def _build(has_b1e: bool, has_b2: bool):
    nc = bass.Bass(target_bir_lowering=False)

    xin = nc.dram_tensor("xin", [L, H], f32, kind="ExternalInput")
    w1g = nc.dram_tensor("w1g", [128, KCH, DF], bf16, kind="ExternalInput")
    w2t = nc.dram_tensor("w2t", [128, FCH, H], bf16, kind="ExternalInput")
    cpb = nc.dram_tensor("cpb", [128, 432], bf16, kind="ExternalInput")
    cpf = nc.dram_tensor("cpf", [128, 8], f32, kind="ExternalInput")
    if has_b2:
        b2r = nc.dram_tensor("b2r", [D, H], bf16, kind="ExternalInput")
    out = nc.dram_tensor("out", [L, H], f32, kind="ExternalOutput")

    # [L, H] viewed as [128p, sub, H] per macro-tile
    x_mt = xin.ap().rearrange("(m s p) h -> m p s h", p=128, s=NSUB)
    out_mt = out.ap().rearrange("(m s p) h -> m p s h", p=128, s=NSUB)

    with tile.TileContext(nc) as tc:
        with (
            tc.tile_pool(name="const", bufs=1) as const,
            tc.tile_pool(name="xp", bufs=2) as xp,
            tc.tile_pool(name="xnp", bufs=2) as xnp,
            tc.tile_pool(name="xtp", bufs=2) as xtp,
            tc.tile_pool(name="midp", bufs=2) as midp,
            tc.tile_pool(name="gmp", bufs=2) as gmp,
            tc.tile_pool(name="gbp", bufs=2) as gbp,
            tc.tile_pool(name="outp", bufs=3) as outp,
            tc.tile_pool(name="smalls", bufs=3) as smalls,
            tc.tile_pool(name="gsm", bufs=2) as gsm,
            tc.tile_pool(name="ps_m1", bufs=2, space="PSUM") as ps_m1,
            tc.tile_pool(name="ps_m2", bufs=2, space="PSUM") as ps_m2,
            tc.tile_pool(name="ps_gux", bufs=1, space="PSUM") as ps_gux,
            tc.tile_pool(name="ps_gv", bufs=1, space="PSUM") as ps_gv,
            tc.tile_pool(name="ps_bc", bufs=2, space="PSUM") as ps_bc,
        ):
            # scratch columns for multiwait carrier ops (dead stores; the
            # first Memset per engine / Copy activation becomes the carrier
            # template in _split_multiwaits)
            scratch = const.tile([128, 4], f32)
            nc.vector.memset(scratch[:, 0:1], 0.0)
            nc.gpsimd.memset(scratch[:, 2:3], 0.0)
            nc.scalar.copy(scratch[0:1, 1:2], scratch[0:1, 3:4])

            # constants on scalar queue (small, needed early); x + weights
            # on sync queue, x sub-tile 0 first, each weight pack one DMA
            # (HWDGE descriptor gen costs ~630ns per DMA instruction)
            cpb_sb = const.tile([128, 432], bf16)
            cpf_sb = const.tile([128, 8], f32)
            nc.scalar.dma_start(out=cpb_sb, in_=cpb.ap())
            nc.scalar.dma_start(out=cpf_sb, in_=cpf.ap())
            oh_sb = cpb_sb[0:D, 0:384]
            gus_sb = cpb_sb[:, 384:408].rearrange("p (k d) -> p k d", d=D)
            w2gv_sb = cpb_sb[:, 408:426].rearrange("p (c d) -> p c d", d=D)
            gusum_sb = cpb_sb[0:1, 426:429]
            ones3_sb = cpb_sb[32:33, 429:432]
            gb3_sb = cpf_sb[0:D, 6:7]

            x_first = xp.tile([128, NSUB, H], f32, tag="x")
            w1g_sb = const.tile([128, KCH, DF], bf16)
            w2t_sb = const.tile([128, FCH, H], bf16)
            nc.sync.dma_start(out=x_first[:, 0, :], in_=x_mt[0][:, 0, :])
            nc.sync.dma_start(out=w1g_sb.rearrange("p a b -> p (a b)"),
                              in_=w1g.ap().rearrange("p a b -> p (a b)"))
            for ss in range(1, NSUB):
                nc.sync.dma_start(out=x_first[:, ss, :], in_=x_mt[0][:, ss, :])
            nc.sync.dma_start(out=w2t_sb.rearrange("p a b -> p (a b)"),
                              in_=w2t.ap().rearrange("p a b -> p (a b)"))
            if has_b2:
                b2r_sb = const.tile([D, H], bf16)
                nc.scalar.dma_start(out=b2r_sb, in_=b2r.ap())

            def stage_a(mt, x_pre=None):
                """x load, stats, normalize (bf16), DMA-transpose."""
                if x_pre is not None:
                    x_t = x_pre
                else:
                    x_t = xp.tile([128, NSUB, H], f32, tag="x")
                    for ss in range(NSUB):
                        nc.sync.dma_start(out=x_t[:, ss, :],
                                          in_=x_mt[mt][:, ss, :])
                xn_b = xnp.tile([128, NSUB, XW], bf16, tag="xn")
                xnT = xtp.tile([128, KCH + 1, T], bf16, tag="xnT")
                for ss in range(NSUB):
                    xs = x_t[:, ss, :]
                    st = smalls.tile([128, 2, 6], f32, tag="bnst")
                    nc.vector.bn_stats(out=st[:, 0, :], in_=xs[:, 0:512])
                    nc.vector.bn_stats(out=st[:, 1, :], in_=xs[:, 512:1024])
                    mv = smalls.tile([128, 2], f32, tag="mv")
                    nc.vector.bn_aggr(out=mv, in_=st)
                    # sc: 0=r=1/s, 1=-mu*r, 2=s=sd+eps
                    sc = smalls.tile([128, 4], f32, tag="sc")
                    nc.scalar.activation(out=sc[:, 2:3], in_=mv[:, 1:2],
                                         func=AF.Sqrt,
                                         scale=float(H) / (H - 1))
                    nc.vector.tensor_scalar_add(sc[:, 2:3], sc[:, 2:3], EPS)
                    nc.vector.reciprocal(sc[:, 0:1], sc[:, 2:3])
                    nc.vector.tensor_scalar(out=sc[:, 1:2], in0=mv[:, 0:1],
                                            scalar1=sc[:, 0:1], scalar2=-1.0,
                                            op0=ALU.mult, op1=ALU.mult)
                    # mu, s ride along in the transpose as bf16 columns;
                    # col H -> row partition 0, col H+32 -> partition 32
                    # (matmul base partitions must be 0/32/64).  Written on
                    # Act so xn_b is single-writer and the transpose DMA
                    # needs no cross-engine wait.
                    nc.scalar.copy(xn_b[:, ss, H:H + 1], mv[:, 0:1])
                    nc.scalar.copy(xn_b[:, ss, H + 32:H + 33], sc[:, 2:3])
                    # xn = x*(1/s) + (-mu/s), one Activation op, bf16 out
                    nc.scalar.activation(out=xn_b[:, ss, 0:H], in_=xs,
                                         func=AF.Identity,
                                         scale=sc[:, 0:1], bias=sc[:, 1:2])
                    nc.scalar.dma_start_transpose(
                        xnT[:, :, ss * 128:(ss + 1) * 128], xn_b[:, ss, :])
                return dict(x_t=x_t, xnT=xnT)

            def stage_b(mt, st_, sliced=False):
                """M1: mid = relu(W1g @ xn^T (+ b1e))."""
                xnT = st_["xnT"]
                mid = midp.tile([128, FCH, T], bf16, tag="mid")
                for c in range(FCH):
                    p1 = ps_m1.tile([128, T], f32, tag="m1")
                    w1c = w1g_sb[:, :, c * 128:(c + 1) * 128]
                    if sliced:
                        for ss in range(NSUB):
                            tsl = slice(ss * 128, (ss + 1) * 128)
                            for k in range(KCH):
                                nc.tensor.matmul(
                                    p1[:, tsl], w1c[:, k, :], xnT[:, k, tsl],
                                    start=(k == 0), stop=(k == KCH - 1))
                    else:
                        for k in range(KCH):
                            nc.tensor.matmul(p1, w1c[:, k, :], xnT[:, k, :],
                                             start=(k == 0),
                                             stop=(k == KCH - 1))
                    bias = cpf_sb[:, c:c + 1] if has_b1e else 0.0
                    nc.scalar.activation(out=mid[:, c, :], in_=p1,
                                         func=AF.Relu, bias=bias)
                st_["mid"] = mid

            def stage_c(mt, st_):
                """Gates + gmid = gate * mid."""
                xnT, mid = st_["xnT"], st_["mid"]
                murow = xnT[0:1, KCH, :]
                srow = xnT[32:33, KCH, :]
                pgux = ps_gux.tile([D, T], f32, tag="gux")
                for k in range(KCH):
                    nc.tensor.matmul(pgux, gus_sb[:, k, :], xnT[:, k, :],
                                     start=(k == 0), stop=(k == KCH - 1))
                pgv = ps_gv.tile([D, T], f32, tag="gv")
                for c in range(FCH):
                    nc.tensor.matmul(pgv, w2gv_sb[:, c, :], mid[:, c, :],
                                     start=(c == 0), stop=False)
                # gu.x = s*(gu.xn) + mu*sum(gu): mu rank-1 joins pgv's psum
                nc.tensor.matmul(pgv, gusum_sb, murow, start=False, stop=True)
                # s broadcast to 3 partitions
                s3_ps = ps_bc.tile([D, T], f32, tag="bc")
                nc.tensor.matmul(s3_ps, ones3_sb, srow, start=True, stop=True)
                s3b = gsm.tile([D, T], bf16, tag="s3")
                nc.scalar.activation(out=s3b, in_=s3_ps, func=AF.Copy)
                z_sb = gsm.tile([D, T], f32, tag="z")
                nc.vector.tensor_tensor(out=z_sb, in0=pgux, in1=s3b,
                                        op=ALU.mult)
                nc.vector.tensor_add(z_sb, z_sb, pgv)
                g_t = gsm.tile([D, T], bf16, tag="g")
                nc.scalar.activation(out=g_t, in_=z_sb, func=AF.Sigmoid,
                                     bias=gb3_sb)
                # broadcast gate row d across partitions via one-hot matmul
                gb128 = gbp.tile([128, D, T], bf16, tag="gb")
                for d in range(D):
                    p_b = ps_bc.tile([128, T], f32, tag="bc")
                    nc.tensor.matmul(p_b, oh_sb[:, d * 128:(d + 1) * 128],
                                     g_t, start=True, stop=True)
                    nc.scalar.activation(out=gb128[:, d, :], in_=p_b,
                                         func=AF.Copy)
                gmid = gmp.tile([128, FCH, T], bf16, tag="gmid")
                for c in range(FCH):
                    nc.vector.tensor_mul(gmid[:, c, :], mid[:, c, :],
                                         gb128[:, c // 2, :])
                st_["gmid"] = gmid
                st_["g_t"] = g_t

            def stage_d(mt, st_):
                """M2 accumulates all domains (+gate*b2) + final out."""
                gmid, x_t = st_["gmid"], st_["x_t"]
                for ss in range(NSUB):
                    tsl = slice(ss * 128, (ss + 1) * 128)
                    out_sb = outp.tile([128, H], f32, tag="osb")
                    for nch in range(NCH):
                        hsl = slice(nch * 512, (nch + 1) * 512)
                        po = ps_m2.tile([128, 512], f32, tag="m2")
                        for c in range(FCH):
                            nc.tensor.matmul(po, gmid[:, c, tsl],
                                             w2t_sb[:, c, hsl],
                                             start=(c == 0),
                                             stop=(c == FCH - 1 and not has_b2))
                        if has_b2:
                            nc.tensor.matmul(po, st_["g_t"][:, tsl],
                                             b2r_sb[:, hsl],
                                             start=False, stop=True)
                        # out = 2*x + pout
                        nc.vector.scalar_tensor_tensor(
                            out=out_sb[:, hsl], in0=x_t[:, ss, hsl],
                            scalar=2.0, in1=po, op0=ALU.mult, op1=ALU.add)
                    # out DMA on the gpsimd (SWDGE) queue: its waits on the
                    # stt then can't block x loads (SP) or Act compute
                    nc.gpsimd.dma_start(out=out_mt[mt][:, ss, :], in_=out_sb)

            # software-pipelined emission: each macro-tile's gate chain
            # overlaps the next tile's M1 in the PE FIFO; mt0's M1 is
            # sliced so the PE starts on the first transposed sub-tile
            S = [None] * NMT
            S[0] = stage_a(0, x_pre=x_first)
            stage_b(0, S[0], sliced=True)
            S[1] = stage_a(1)
            stage_c(0, S[0])
            stage_b(1, S[1])
            stage_d(0, S[0])
            S[2] = stage_a(2)
            stage_c(1, S[1])
            stage_b(2, S[2])
            stage_d(1, S[1])
            S[3] = stage_a(3)
            stage_c(2, S[2])
            stage_b(3, S[3])
            stage_d(2, S[2])
            stage_c(3, S[3])
            stage_d(3, S[3])

    _split_multiwaits(nc)
    return nc


last_results = None

_built = {}


def _get_nc(has_b1e, has_b2):
    key = (has_b1e, has_b2)
    if key not in _built:
        _built[key] = _build(*key)
    return _built[key]


def _to_bf16(a):
    from ml_dtypes import bfloat16
    return np.asarray(a, dtype=np.float32).astype(bfloat16)


def kernel(x, ln_g, ln_b, W1, b1, W2, b2, gu, gv, gb):
    x = np.asarray(x, dtype=np.float32)
    ln_g = np.asarray(ln_g, dtype=np.float32)
    ln_b = np.asarray(ln_b, dtype=np.float32)
    W1 = np.asarray(W1, dtype=np.float32)
    b1 = np.asarray(b1, dtype=np.float32)
    W2 = np.asarray(W2, dtype=np.float32)
    b2 = np.asarray(b2, dtype=np.float32)
    gu = np.asarray(gu, dtype=np.float32)
    gv = np.asarray(gv, dtype=np.float32)
    gb = np.asarray(gb, dtype=np.float32)

    # ---- host precompute (all small: ~D*F*H) ----
    W1G = W1 * ln_g[:, None, :]                                # [D, F, H]
    b1e = b1 + np.einsum('dfh,dh->df', W1, ln_b)               # [D, F]
    w2gv = np.einsum('dh,dhf->df', gv, W2)                     # [D, F]
    gusum = gu.sum(axis=1)                                     # [D]
    gb_eff = gb + np.einsum('dh,dh->d', gv, b2)                # [D]

    has_b1e = bool(np.any(b1e != 0.0))
    has_b2 = bool(np.any(b2 != 0.0))

    # lhsT for M1: [128, KCH, DF]; col c*128+j = W1G[d(c), fh(c)*128+j, h]
    w1g_in = np.zeros((128, KCH, DF), dtype=np.float32)
    for c in range(FCH):
        d, fh = c // 2, c % 2
        w1g_in[:, :, c * 128:(c + 1) * 128] = (
            W1G[d].T.reshape(KCH, 128, F)[:, :, fh * 128:(fh + 1) * 128]
            .transpose(1, 0, 2))
    # W2 rhs for M2: [128, FCH, H]; w2t[p, c, h] = W2[d, h, fh*128+p]
    w2t_in = np.zeros((128, FCH, H), dtype=np.float32)
    for c in range(FCH):
        d, fh = c // 2, c % 2
        w2t_in[:, c, :] = W2[d, :, fh * 128:(fh + 1) * 128].T

    cpb_in = np.zeros((128, 432), dtype=np.float32)
    for d in range(D):
        cpb_in[d, d * 128:(d + 1) * 128] = 1.0                 # one-hot bcast
    cpb_in[:, 384:408] = np.ascontiguousarray(
        gu.T.reshape(KCH, 128, D).transpose(1, 0, 2)).reshape(128, KCH * D)
    w2gv_in = np.zeros((128, FCH, D), dtype=np.float32)
    for c in range(FCH):
        d, fh = c // 2, c % 2
        w2gv_in[:, c, d] = w2gv[d, fh * 128:(fh + 1) * 128]
    cpb_in[:, 408:426] = w2gv_in.reshape(128, FCH * D)
    cpb_in[0, 426:429] = gusum
    cpb_in[32, 429:432] = 1.0

    cpf_in = np.zeros((128, 8), dtype=np.float32)
    if has_b1e:
        for c in range(FCH):
            d, fh = c // 2, c % 2
            cpf_in[:, c] = b1e[d, fh * 128:(fh + 1) * 128]
    cpf_in[0:D, 6] = gb_eff

    nc = _get_nc(has_b1e, has_b2)

    common = {
        "w1g": _to_bf16(w1g_in),
        "w2t": _to_bf16(w2t_in),
        "cpb": _to_bf16(cpb_in),
        "cpf": cpf_in,
    }
    if has_b2:
        common["b2r"] = _to_bf16(b2)
    in_maps = [dict(common, xin=np.ascontiguousarray(x[c]))
               for c in range(B)]
    res = run_bass_kernel_spmd(nc, in_maps, core_ids=list(range(B)))
    global last_results
    last_results = res
    return np.stack([res.results[c]["out"] for c in range(B)])


# revision 18
# speedup vs baseline: 1.1581x; 1.1140x over previous
"""Trainium2 Bass kernel for nn_MixtureOfAdapter (moe_routing).

Math (per token, H=1024, F=256, D=3 domains):
    mu, sd (ddof=1) over H;  s = sd + eps;  xn = (x - mu)/s
    h_d   = xn*g_d + b_d
    mid_d = relu(W1_d h_d + b1_d);  a_d = W2_d mid_d + b2_d
    gate_d = sigmoid(gu_d.x + gv_d.a_d + gb_d)
    out = 2x + sum_d gate_d * a_d

Kernel strategy (8 cores, data-parallel over batch B=8):
  - All matmul-land tensors are bf16: transposes and matmuls run at
    1 cyc/row on the PE, and weights/activations halve SBUF + DMA.
  - Work in normalized-transposed land: per 512-token macro-tile the
    centered/normalized xn = (x-mu)/s (computed by one Activation
    Identity op with per-partition scale=1/s, bias=-mu/s) is moved to
    [h, t] layout by the DMA XBAR transpose (dma_start_transpose), not
    the PE.  Two extra bf16 columns (mu, s) ride along in the same
    transpose and come out as [1, t] rows for rank-1 corrections.
  - M1: mid = relu(W1g @ xn^T (+ b1e per-partition bias)) with
    W1g = W1 * ln_g folded host-side.  True mid (no s scaling).
  - Gates: pgv[d,t] = w2gv_d . mid_d (+ mu-row rank-1 for gu.x's mean
    term); pgux[d,t] = gu_d . xn^T; z = pgux*s + pgv;
    gate = sigmoid(z + (gb_d + gv_d.b2_d)).  s broadcast to 3
    partitions via a ones3 rank-1 matmul.
  - gate rows broadcast to 128 partitions via one-hot matmuls; gmid =
    mid * gate (bf16, 2x DVE); M2 accumulates all domains into one
    PSUM in natural [t, h] layout (+ gate-row rank-1 if b2 nonzero);
    out = 2x + pout via one DVE scalar_tensor_tensor per 512-chunk.
  - Software-pipelined emission keeps each macro-tile's gate chain
    (DVE/Act latency) hidden behind the next tile's M1 in the PE FIFO;
    macro-tile 0's M1 is emitted in 128-token slices so the PE starts
    as soon as the first sub-tile's transpose lands.
"""

import numpy as np

import concourse.bass as bass
import concourse.mybir as mybir
import concourse.tile as tile
from concourse.bass_utils import run_bass_kernel_spmd

B, L, H, F, D = 8, 2048, 1024, 256, 3
EPS = 1e-6
T = 512                 # tokens per macro-tile
NSUB = T // 128         # 4 sub-tiles of 128 tokens
NMT = L // T            # 4 macro-tiles per core
KCH = H // 128          # 8 k-chunks over H
FCH = (D * F) // 128    # 6 chunks over stacked (domain, F)
NCH = H // 512          # 2 output column chunks
DF = D * F
XW = H + 128            # transpose width: H cols + (mu, s, pad) block

f32 = mybir.dt.float32
bf16 = mybir.dt.bfloat16
AF = mybir.ActivationFunctionType
ALU = mybir.AluOpType


def _split_multiwaits(nc):
    """This walrus build allows 1 sync-wait per instruction (2 for
    EventSemaphore); Tile can attach more.  Move extras onto preceding
    same-engine carrier instructions.  A bare NoOp holds the sequencer
    while it waits (stalling dispatch of everything behind it), so where
    possible the carrier is a 1-element Memset to a dead scratch column:
    a real engine instruction parks its wait in the engine wait queue
    and lets the sequencer keep dispatching."""
    import copy
    tmpl = {}
    for f in nc.m.functions:
        for bb in f.blocks:
            for inst in bb.instructions:
                if (isinstance(inst, mybir.InstMemset)
                        and inst.engine not in tmpl):
                    tmpl[inst.engine] = inst
                elif (isinstance(inst, mybir.InstActivation)
                        and inst.func == AF.Copy
                        and inst.engine not in tmpl):
                    tmpl[inst.engine] = inst

    def carrier(inst, w, j):
        t = tmpl.get(inst.engine)
        if t is not None:
            c = copy.deepcopy(t)
            c.name = f"{inst.name}-wsplit{j}"
            c.sync_info = mybir.SyncInfo(on_wait=[w], on_update=[])
            return c
        return mybir.InstNoOp(
            name=f"{inst.name}-wsplit{j}",
            engine=inst.engine,
            sync_info=mybir.SyncInfo(on_wait=[w], on_update=[]),
            ins=[], outs=[],
        )

    for f in nc.m.functions:
        for bb in f.blocks:
            new = []
            changed = False
            for inst in bb.instructions:
                si = inst.sync_info
                cap = 2 if isinstance(inst, mybir.InstEventSemaphore) else 1
                if si is not None and len(si.on_wait) > cap:
                    waits = list(si.on_wait)
                    extra, kept = waits[:-cap], waits[-cap:]
                    for j, w in enumerate(extra):
                        new.append(carrier(inst, w, j))
                    inst.sync_info = mybir.SyncInfo(
                        on_wait=kept, on_update=list(si.on_update))
                    changed = True
                new.append(inst)
            if changed:
                bb.instructions = new


def _build(has_b1e: bool, has_b2: bool):
    nc = bass.Bass(target_bir_lowering=False)

    xin = nc.dram_tensor("xin", [L, H], f32, kind="ExternalInput")
    w1g = nc.dram_tensor("w1g", [128, KCH, DF], bf16, kind="ExternalInput")
    w2t = nc.dram_tensor("w2t", [128, FCH, H], bf16, kind="ExternalInput")
    cpb = nc.dram_tensor("cpb", [128, 432], bf16, kind="ExternalInput")
    cpf = nc.dram_tensor("cpf", [128, 8], f32, kind="ExternalInput")
    if has_b2:
        b2r = nc.dram_tensor("b2r", [D, H], bf16, kind="ExternalInput")
    out = nc.dram_tensor("out", [L, H], f32, kind="ExternalOutput")

    # [L, H] viewed as [128p, sub, H] per macro-tile
    x_mt = xin.ap().rearrange("(m s p) h -> m p s h", p=128, s=NSUB)
    out_mt = out.ap().rearrange("(m s p) h -> m p s h", p=128, s=NSUB)

    with tile.TileContext(nc) as tc:
        with (
            tc.tile_pool(name="const", bufs=1) as const,
            tc.tile_pool(name="xp", bufs=2) as xp,
            tc.tile_pool(name="xnp", bufs=2) as xnp,
            tc.tile_pool(name="xtp", bufs=2) as xtp,
            tc.tile_pool(name="midp", bufs=2) as midp,
            tc.tile_pool(name="gmp", bufs=2) as gmp,
            tc.tile_pool(name="gbp", bufs=2) as gbp,
            tc.tile_pool(name="outp", bufs=3) as outp,
            tc.tile_pool(name="smalls", bufs=3) as smalls,
            tc.tile_pool(name="gsm", bufs=2) as gsm,
            tc.tile_pool(name="ps_m1", bufs=2, space="PSUM") as ps_m1,
            tc.tile_pool(name="ps_m2", bufs=2, space="PSUM") as ps_m2,
            tc.tile_pool(name="ps_gux", bufs=1, space="PSUM") as ps_gux,
            tc.tile_pool(name="ps_gv", bufs=1, space="PSUM") as ps_gv,
            tc.tile_pool(name="ps_bc", bufs=2, space="PSUM") as ps_bc,
        ):
            # scratch columns for multiwait carrier ops (dead stores; the
            # first Memset per engine / Copy activation becomes the carrier
            # template in _split_multiwaits)
            scratch = const.tile([128, 4], f32)
            nc.vector.memset(scratch[:, 0:1], 0.0)
            nc.gpsimd.memset(scratch[:, 2:3], 0.0)
            nc.scalar.copy(scratch[0:1, 1:2], scratch[0:1, 3:4])

            # constants on scalar queue (small, needed early); x + weights
            # on sync queue, x sub-tile 0 first, each weight pack one DMA
            # (HWDGE descriptor gen costs ~630ns per DMA instruction)
            cpb_sb = const.tile([128, 432], bf16)
            cpf_sb = const.tile([128, 8], f32)
            nc.scalar.dma_start(out=cpb_sb, in_=cpb.ap())
            nc.scalar.dma_start(out=cpf_sb, in_=cpf.ap())
            oh_sb = cpb_sb[0:D, 0:384]
            gus_sb = cpb_sb[:, 384:408].rearrange("p (k d) -> p k d", d=D)
            w2gv_sb = cpb_sb[:, 408:426].rearrange("p (c d) -> p c d", d=D)
            gusum_sb = cpb_sb[0:1, 426:429]
            ones3_sb = cpb_sb[32:33, 429:432]
            gb3_sb = cpf_sb[0:D, 6:7]

            x_first = xp.tile([128, NSUB, H], f32, tag="x")
            w1g_sb = const.tile([128, KCH, DF], bf16)
            w2t_sb = const.tile([128, FCH, H], bf16)
            nc.sync.dma_start(out=x_first[:, 0, :], in_=x_mt[0][:, 0, :])
            nc.sync.dma_start(out=w1g_sb.rearrange("p a b -> p (a b)"),
                              in_=w1g.ap().rearrange("p a b -> p (a b)"))
            for ss in range(1, NSUB):
                nc.sync.dma_start(out=x_first[:, ss, :], in_=x_mt[0][:, ss, :])
            nc.sync.dma_start(out=w2t_sb.rearrange("p a b -> p (a b)"),
                              in_=w2t.ap().rearrange("p a b -> p (a b)"))
            if has_b2:
                b2r_sb = const.tile([D, H], bf16)
                nc.scalar.dma_start(out=b2r_sb, in_=b2r.ap())

            def stage_a(mt, x_pre=None):
                """x load, stats, normalize (bf16), DMA-transpose."""
                if x_pre is not None:
                    x_t = x_pre
                else:
                    # steady-state x loads ride the SWDGE (Pool) DMA lanes
                    # so they never share a completion-sem lane with the
                    # compute-gated transpose DMAs (8-lane round-robin)
                    x_t = xp.tile([128, NSUB, H], f32, tag="x")
                    for ss in range(NSUB):
                        nc.gpsimd.dma_start(out=x_t[:, ss, :],
                                            in_=x_mt[mt][:, ss, :])
                xn_b = xnp.tile([128, NSUB, XW], bf16, tag="xn")
                xnT = xtp.tile([128, KCH + 1, T], bf16, tag="xnT")
                for ss in range(NSUB):
                    xs = x_t[:, ss, :]
                    st = smalls.tile([128, 2, 6], f32, tag="bnst")
                    nc.vector.bn_stats(out=st[:, 0, :], in_=xs[:, 0:512])
                    nc.vector.bn_stats(out=st[:, 1, :], in_=xs[:, 512:1024])
                    mv = smalls.tile([128, 2], f32, tag="mv")
                    nc.vector.bn_aggr(out=mv, in_=st)
                    # sc: 0=r=1/s, 1=-mu*r, 2=s=sd+eps
                    sc = smalls.tile([128, 4], f32, tag="sc")
                    nc.scalar.activation(out=sc[:, 2:3], in_=mv[:, 1:2],
                                         func=AF.Sqrt,
                                         scale=float(H) / (H - 1))
                    nc.vector.tensor_scalar_add(sc[:, 2:3], sc[:, 2:3], EPS)
                    nc.vector.reciprocal(sc[:, 0:1], sc[:, 2:3])
                    nc.vector.tensor_scalar(out=sc[:, 1:2], in0=mv[:, 0:1],
                                            scalar1=sc[:, 0:1], scalar2=-1.0,
                                            op0=ALU.mult, op1=ALU.mult)
                    # mu, s ride along in the transpose as bf16 columns;
                    # col H -> row partition 0, col H+32 -> partition 32
                    # (matmul base partitions must be 0/32/64).  Written on
                    # Act so xn_b is single-writer and the transpose DMA
                    # needs no cross-engine wait.
                    nc.scalar.copy(xn_b[:, ss, H:H + 1], mv[:, 0:1])
                    nc.scalar.copy(xn_b[:, ss, H + 32:H + 33], sc[:, 2:3])
                    # xn = x*(1/s) + (-mu/s), one Activation op, bf16 out
                    nc.scalar.activation(out=xn_b[:, ss, 0:H], in_=xs,
                                         func=AF.Identity,
                                         scale=sc[:, 0:1], bias=sc[:, 1:2])
                    # transpose DMA on the otherwise-idle sync queue: its
                    # wait on the center op then can't stall Act dispatch
                    nc.sync.dma_start_transpose(
                        xnT[:, :, ss * 128:(ss + 1) * 128], xn_b[:, ss, :])
                return dict(x_t=x_t, xnT=xnT)

            def stage_b(mt, st_, sliced=False):
                """M1: mid = relu(W1g @ xn^T (+ b1e))."""
                xnT = st_["xnT"]
                mid = midp.tile([128, FCH, T], bf16, tag="mid")
                for c in range(FCH):
                    p1 = ps_m1.tile([128, T], f32, tag="m1")
                    w1c = w1g_sb[:, :, c * 128:(c + 1) * 128]
                    if sliced:
                        for ss in range(NSUB):
                            tsl = slice(ss * 128, (ss + 1) * 128)
                            for k in range(KCH):
                                nc.tensor.matmul(
                                    p1[:, tsl], w1c[:, k, :], xnT[:, k, tsl],
                                    start=(k == 0), stop=(k == KCH - 1))
                    else:
                        for k in range(KCH):
                            nc.tensor.matmul(p1, w1c[:, k, :], xnT[:, k, :],
                                             start=(k == 0),
                                             stop=(k == KCH - 1))
                    bias = cpf_sb[:, c:c + 1] if has_b1e else 0.0
                    nc.scalar.activation(out=mid[:, c, :], in_=p1,
                                         func=AF.Relu, bias=bias)
                st_["mid"] = mid

            def stage_c(mt, st_):
                """Gates + gmid = gate * mid."""
                xnT, mid = st_["xnT"], st_["mid"]
                murow = xnT[0:1, KCH, :]
                srow = xnT[32:33, KCH, :]
                pgux = ps_gux.tile([D, T], f32, tag="gux")
                for k in range(KCH):
                    nc.tensor.matmul(pgux, gus_sb[:, k, :], xnT[:, k, :],
                                     start=(k == 0), stop=(k == KCH - 1))
                pgv = ps_gv.tile([D, T], f32, tag="gv")
                for c in range(FCH):
                    nc.tensor.matmul(pgv, w2gv_sb[:, c, :], mid[:, c, :],
                                     start=(c == 0), stop=False)
                # gu.x = s*(gu.xn) + mu*sum(gu): mu rank-1 joins pgv's psum
                nc.tensor.matmul(pgv, gusum_sb, murow, start=False, stop=True)
                # s broadcast to 3 partitions
                s3_ps = ps_bc.tile([D, T], f32, tag="bc")
                nc.tensor.matmul(s3_ps, ones3_sb, srow, start=True, stop=True)
                s3b = gsm.tile([D, T], bf16, tag="s3")
                nc.scalar.activation(out=s3b, in_=s3_ps, func=AF.Copy)
                z_sb = gsm.tile([D, T], f32, tag="z")
                nc.vector.tensor_tensor(out=z_sb, in0=pgux, in1=s3b,
                                        op=ALU.mult)
                nc.vector.tensor_add(z_sb, z_sb, pgv)
                g_t = gsm.tile([D, T], bf16, tag="g")
                nc.scalar.activation(out=g_t, in_=z_sb, func=AF.Sigmoid,
                                     bias=gb3_sb)
                # broadcast gate row d across partitions via one-hot matmul
                gb128 = gbp.tile([128, D, T], bf16, tag="gb")
                for d in range(D):
                    p_b = ps_bc.tile([128, T], f32, tag="bc")
                    nc.tensor.matmul(p_b, oh_sb[:, d * 128:(d + 1) * 128],
                                     g_t, start=True, stop=True)
                    nc.scalar.activation(out=gb128[:, d, :], in_=p_b,
                                         func=AF.Copy)
                gmid = gmp.tile([128, FCH, T], bf16, tag="gmid")
                for c in range(FCH):
                    nc.vector.tensor_mul(gmid[:, c, :], mid[:, c, :],
                                         gb128[:, c // 2, :])
                st_["gmid"] = gmid
                st_["g_t"] = g_t

            def stage_d(mt, st_):
                """M2 accumulates all domains (+gate*b2) + final out."""
                gmid, x_t = st_["gmid"], st_["x_t"]
                for ss in range(NSUB):
                    tsl = slice(ss * 128, (ss + 1) * 128)
                    out_sb = outp.tile([128, H], f32, tag="osb")
                    for nch in range(NCH):
                        hsl = slice(nch * 512, (nch + 1) * 512)
                        po = ps_m2.tile([128, 512], f32, tag="m2")
                        for c in range(FCH):
                            nc.tensor.matmul(po, gmid[:, c, tsl],
                                             w2t_sb[:, c, hsl],
                                             start=(c == 0),
                                             stop=(c == FCH - 1 and not has_b2))
                        if has_b2:
                            nc.tensor.matmul(po, st_["g_t"][:, tsl],
                                             b2r_sb[:, hsl],
                                             start=False, stop=True)
                        # out = 2*x + pout
                        nc.vector.scalar_tensor_tensor(
                            out=out_sb[:, hsl], in0=x_t[:, ss, hsl],
                            scalar=2.0, in1=po, op0=ALU.mult, op1=ALU.add)
                    # out DMA on the gpsimd (SWDGE) queue: its waits on the
                    # stt then can't block x loads (SP) or Act compute
                    nc.gpsimd.dma_start(out=out_mt[mt][:, ss, :], in_=out_sb)

            # software-pipelined emission: each macro-tile's gate chain
            # overlaps the next tile's M1 in the PE FIFO; mt0's M1 is
            # sliced so the PE starts on the first transposed sub-tile
            S = [None] * NMT
            S[0] = stage_a(0, x_pre=x_first)
            stage_b(0, S[0], sliced=True)
            S[1] = stage_a(1)
            stage_c(0, S[0])
            stage_b(1, S[1])
            S[2] = stage_a(2)
            stage_d(0, S[0])
            stage_c(1, S[1])
            stage_b(2, S[2])
            S[3] = stage_a(3)
            stage_d(1, S[1])
            stage_c(2, S[2])
            stage_b(3, S[3])
            stage_d(2, S[2])
            stage_c(3, S[3])
            stage_d(3, S[3])

    _split_multiwaits(nc)
    return nc


last_results = None

_built = {}


def _get_nc(has_b1e, has_b2):
    key = (has_b1e, has_b2)
    if key not in _built:
        _built[key] = _build(*key)
    return _built[key]


def _to_bf16(a):
    from ml_dtypes import bfloat16
    return np.asarray(a, dtype=np.float32).astype(bfloat16)


def kernel(x, ln_g, ln_b, W1, b1, W2, b2, gu, gv, gb):
    x = np.asarray(x, dtype=np.float32)
    ln_g = np.asarray(ln_g, dtype=np.float32)
    ln_b = np.asarray(ln_b, dtype=np.float32)
    W1 = np.asarray(W1, dtype=np.float32)
    b1 = np.asarray(b1, dtype=np.float32)
    W2 = np.asarray(W2, dtype=np.float32)
    b2 = np.asarray(b2, dtype=np.float32)
    gu = np.asarray(gu, dtype=np.float32)
    gv = np.asarray(gv, dtype=np.float32)
    gb = np.asarray(gb, dtype=np.float32)

    # ---- host precompute (all small: ~D*F*H) ----
    W1G = W1 * ln_g[:, None, :]                                # [D, F, H]
    b1e = b1 + np.einsum('dfh,dh->df', W1, ln_b)               # [D, F]
    w2gv = np.einsum('dh,dhf->df', gv, W2)                     # [D, F]
    gusum = gu.sum(axis=1)                                     # [D]
    gb_eff = gb + np.einsum('dh,dh->d', gv, b2)                # [D]

    has_b1e = bool(np.any(b1e != 0.0))
    has_b2 = bool(np.any(b2 != 0.0))

    # lhsT for M1: [128, KCH, DF]; col c*128+j = W1G[d(c), fh(c)*128+j, h]
    w1g_in = np.zeros((128, KCH, DF), dtype=np.float32)
    for c in range(FCH):
        d, fh = c // 2, c % 2
        w1g_in[:, :, c * 128:(c + 1) * 128] = (
            W1G[d].T.reshape(KCH, 128, F)[:, :, fh * 128:(fh + 1) * 128]
            .transpose(1, 0, 2))
    # W2 rhs for M2: [128, FCH, H]; w2t[p, c, h] = W2[d, h, fh*128+p]
    w2t_in = np.zeros((128, FCH, H), dtype=np.float32)
    for c in range(FCH):
        d, fh = c // 2, c % 2
        w2t_in[:, c, :] = W2[d, :, fh * 128:(fh + 1) * 128].T

    cpb_in = np.zeros((128, 432), dtype=np.float32)
    for d in range(D):
        cpb_in[d, d * 128:(d + 1) * 128] = 1.0                 # one-hot bcast
    cpb_in[:, 384:408] = np.ascontiguousarray(
        gu.T.reshape(KCH, 128, D).transpose(1, 0, 2)).reshape(128, KCH * D)
    w2gv_in = np.zeros((128, FCH, D), dtype=np.float32)
    for c in range(FCH):
        d, fh = c // 2, c % 2
        w2gv_in[:, c, d] = w2gv[d, fh * 128:(fh + 1) * 128]
    cpb_in[:, 408:426] = w2gv_in.reshape(128, FCH * D)
    cpb_in[0, 426:429] = gusum
    cpb_in[32, 429:432] = 1.0

    cpf_in = np.zeros((128, 8), dtype=np.float32)
    if has_b1e:
        for c in range(FCH):
            d, fh = c // 2, c % 2
            cpf_in[:, c] = b1e[d, fh * 128:(fh + 1) * 128]
    cpf_in[0:D, 6] = gb_eff

    nc = _get_nc(has_b1e, has_b2)

    common = {
        "w1g": _to_bf16(w1g_in),
        "w2t": _to_bf16(w2t_in),
        "cpb": _to_bf16(cpb_in),
        "cpf": cpf_in,
    }
    if has_b2:
        common["b2r"] = _to_bf16(b2)
    in_maps = [dict(common, xin=np.ascontiguousarray(x[c]))
               for c in range(B)]
    res = run_bass_kernel_spmd(nc, in_maps, core_ids=list(range(B)))
    global last_results
    last_results = res
    return np.stack([res.results[c]["out"] for c in range(B)])
